# revision 16
# baseline (speedup 1.0000x reference)
"""LMU-FFT cell as a Bass/Tile kernel on 8 trn2 NeuronCores.

Replaces the reference FFT convolution with an exact chunked state-space
form (validated to ~3e-7 rel err vs the FFT on CPU):
  u = relu(x @ Wu^T + b)                          [B, S]
  chunks of Lt=32: p[c] = Hrev @ u_c; boundary states via log-depth
  matrix scan s[c] = sum_d Ad^(32(c-d)) p[d]; then
  m[c] = W_big^T @ [s[c-1]; u_c]  where W_big packs Ad^(i+1) and the
  local Toeplitz H columns; finally h = relu([m, x] @ Wh^T + bias).
Data-parallel over batch: 2 batch rows per core, all params replicated.
"""
import numpy as np

B, S, I, MEM, HID = 16, 4096, 256, 256, 512
LT = 32
C = S // LT            # 128 chunks per sequence
NB = 2                 # batches per core
NCORES = 8
THETA = float(S)

_CACHE = {}
CFG = {"pt": 2, "pm": 3, "pf": 3, "wp": 8, "w2": 4, "xp": 6, "hp": 6, "sp": 4, "mergeevac": True, "actsplit": False}


def _state_space_f32():
    import jax
    import jax.numpy as jnp
    from jax.scipy.linalg import expm
    n = MEM
    Q = np.arange(n, dtype=np.float64).reshape(-1, 1)
    R = (2 * Q + 1) / THETA
    i, j = np.meshgrid(Q, Q, indexing='ij')
    A = R * np.where(i < j, -1.0, (-1.0) ** (i - j + 1))
    Bm = R * (-1.0) ** Q
    M = np.zeros((n + 1, n + 1), dtype=np.float32)
    M[:n, :n] = A.astype(np.float32)
    M[:n, n:] = Bm.astype(np.float32)
    with jax.default_device(jax.devices('cpu')[0]):
        Md = np.array(expm(jnp.asarray(M)))
    return Md[:n, :n]


def _host_consts(H):
    Ad64 = _state_space_f32().astype(np.float64)
    G = np.empty((LT, MEM, MEM), dtype=np.float64)
    P = Ad64.copy()
    for i in range(LT):
        if i > 0:
            P = P @ Ad64
        G[i] = P
    scanT = np.empty((7, MEM, MEM), dtype=np.float32)
    Q = np.linalg.matrix_power(Ad64, LT)
    for r in range(7):
        scanT[r] = Q.T.astype(np.float32)
        Q = Q @ Q
    W_big = np.zeros((MEM + LT, LT * MEM), dtype=np.float32)
    for i in range(LT):
        W_big[:MEM, i * MEM:(i + 1) * MEM] = G[i].T.astype(np.float32)
        for jj in range(i + 1):
            W_big[MEM + jj, i * MEM:(i + 1) * MEM] = H[:, i - jj]
    Hrev32 = np.ascontiguousarray(H[:, :LT][:, ::-1].T)   # [LT, MEM]
    return W_big, scanT, Hrev32


def _build_program(max_phase=99):
    import concourse.bass as bass
    import concourse.mybir as mybir
    import concourse.tile as tile
    from concourse import bacc
    from concourse.masks import make_identity

    f32 = mybir.dt.float32
    f32r = mybir.dt.float32r
    USE_F32R = True

    def rr(ap):
        return ap.bitcast(f32r) if USE_F32R else ap
    nc = bacc.Bacc("TRN2", target_bir_lowering=False, debug=False)

    x_in = nc.declare_dram_parameter("x_in", [NB, S, I], f32, isOutput=False)
    wbig_in = nc.declare_dram_parameter("wbig", [MEM + LT, LT * MEM], f32, isOutput=False)
    scan_in = nc.declare_dram_parameter("scan7", [7, MEM, MEM], f32, isOutput=False)
    hrev_in = nc.declare_dram_parameter("hrev", [LT, MEM], f32, isOutput=False)
    whcat_in = nc.declare_dram_parameter("whcat", [HID, HID], f32, isOutput=False)
    bias_in = nc.declare_dram_parameter("biasb", [128, HID], f32, isOutput=False)
    wu_in = nc.declare_dram_parameter("wu", [I, 1], f32, isOutput=False)
    wub_in = nc.declare_dram_parameter("wub", [1, 1], f32, isOutput=False)
    h_out = nc.declare_dram_parameter("h_out", [NB, S, HID], f32, isOutput=True)
    u_dram = nc.dram_tensor("u_scratch", [NB, S], f32)

    TT = S // 128        # 32 time-tiles per batch

    with tile.TileContext(nc) as tc:
        with (
            tc.tile_pool(name="const", bufs=1) as cpool,
            tc.tile_pool(name="big", bufs=1) as bigpool,
            tc.tile_pool(name="xin", bufs=CFG["xp"]) as xpool,
            tc.tile_pool(name="wbig", bufs=CFG["wp"]) as wpool,
            tc.tile_pool(name="wbig2", bufs=CFG["w2"]) as w2pool,
            tc.tile_pool(name="scanw", bufs=CFG["sp"]) as spool,
            tc.tile_pool(name="hout", bufs=CFG["hp"]) as hpool,
            tc.tile_pool(name="utmp", bufs=4) as upool,
            tc.tile_pool(name="pt", bufs=CFG["pt"], space="PSUM") as pt,
            tc.tile_pool(name="pm", bufs=CFG["pm"], space="PSUM") as pm,
            tc.tile_pool(name="pf", bufs=CFG["pf"], space="PSUM") as pf,
        ):
            # ---- constants in SBUF ----
            ident = cpool.tile([128, 128], f32, tag="ident")
            make_identity(nc, ident[:])
            hrev_sb = cpool.tile([LT, MEM], f32, tag="hrev")
            nc.sync.dma_start(rr(hrev_sb[:]), rr(hrev_in[:]))
            wh_sb = [cpool.tile([128, HID], f32, tag=f"wh{k}", name=f"wh{k}") for k in range(4)]
            for k in range(4):
                nc.sync.dma_start(rr(wh_sb[k][:]), rr(whcat_in[k * 128:(k + 1) * 128, :]))
            bias_sb = cpool.tile([128, HID], f32, tag="bias")
            nc.sync.dma_start(bias_sb[:], bias_in[:])
            wu_sb = [cpool.tile([128, 1], f32, tag=f"wu{k}", name=f"wu{k}") for k in range(2)]
            for k in range(2):
                nc.sync.dma_start(rr(wu_sb[k][:]), rr(wu_in[k * 128:(k + 1) * 128, :]))
            wub_sb = cpool.tile([1, 1], f32, tag="wub")
            nc.sync.dma_start(wub_sb[:], wub_in[:])

            # persistent big buffers
            xT = [[bigpool.tile([128, S], f32, tag=f"xT{b}{fh}", name=f"xT{b}{fh}") for fh in range(2)]
                  for b in range(NB)]
            mT = [[bigpool.tile([128, S], f32, tag=f"mT{b}{kh}", name=f"mT{b}{kh}") for kh in range(2)]
                  for b in range(NB)] if not CFG["mergeevac"] else None
            mTj = [bigpool.tile([128, NB * S], f32, tag=f"mTj{kh}", name=f"mTj{kh}") for kh in range(2)] if CFG["mergeevac"] else None
            sT = [bigpool.tile([128, NB * C], f32, tag=f"sT{k}", name=f"sT{k}") for k in range(2)]
            sT2 = [bigpool.tile([128, NB * C], f32, tag=f"sT2{k}", name=f"sT2{k}") for k in range(2)]
            sTx = [bigpool.tile([128, NB * C], f32, tag=f"sTx{k}", name=f"sTx{k}") for k in range(2)]
            uT = bigpool.tile([LT, NB * C], f32, tag="uT")
            ucj = [bigpool.tile([128, LT], f32, tag=f"ucj{b}", name=f"ucj{b}") for b in range(NB)]

            # ---- phase 1: load x, build xT via PE transpose ----
            for b in range(NB):
                for t in range(TT):
                    xt = xpool.tile([128, I], f32, tag="xin")
                    nc.sync.dma_start(xt[:], x_in[b, t * 128:(t + 1) * 128, :])
                    for fh in range(2):
                        ps = pt.tile([128, 128], f32, tag="pt")
                        nc.tensor.transpose(ps[:], xt[:, fh * 128:(fh + 1) * 128], ident[:])
                        if CFG["actsplit"] and fh == 1:
                            nc.scalar.copy(rr(xT[b][fh][:, t * 128:(t + 1) * 128]), ps[:])
                        else:
                            nc.vector.tensor_copy(rr(xT[b][fh][:, t * 128:(t + 1) * 128]), ps[:])

            # ---- phase 2: u = relu(x @ Wu + b), reshape to uT [32, 256] ----
            for b in range(NB if max_phase >= 2 else 0):
                for q in range(S // 512):
                    pu = pf.tile([1, 512], f32, tag="pf")
                    for fh in range(2):
                        nc.tensor.matmul(
                            pu[:], wu_sb[fh][:].bitcast(f32r),
                            xT[b][fh][:, q * 512:(q + 1) * 512].bitcast(f32r),
                            start=(fh == 0), stop=(fh == 1))
                    ut = upool.tile([1, 512], f32, tag="utmp")
                    nc.scalar.activation(ut[:], pu[:],
                                         mybir.ActivationFunctionType.Relu,
                                         bias=wub_sb[0:1, 0:1])
                    nc.sync.dma_start(u_dram[b, q * 512:(q + 1) * 512].rearrange("(a w) -> a w", a=1), ut[0:1, :])
                for _ in range(1):
                    nc.sync.dma_start(
                        ucj[b][:], u_dram[b, :].rearrange("(c j) -> c j", j=LT))
                ps = pt.tile([128, 128], f32, tag="pt")
                nc.tensor.transpose(ps[0:LT, 0:128], ucj[b][:], ident[:])
                nc.vector.tensor_copy(rr(uT[:, b * C:(b + 1) * C]), ps[0:LT, 0:128])

            # ---- phase 3: p init: sT[kh] = Hrev^T @ uT ----
            for kh in range(2 if max_phase >= 3 else 0):
                ps = pm.tile([128, NB * C], f32, tag="pm")
                nc.tensor.matmul(ps[:], hrev_sb[:, kh * 128:(kh + 1) * 128].bitcast(f32r),
                                 uT[:].bitcast(f32r), start=True, stop=True)
                nc.vector.tensor_copy(rr(sT[kh][:]), ps[:])

            # ---- phase 4: Hillis-Steele scan over chunks (7 rounds) ----
            for r in range(7 if max_phase >= 4 else 0):
                sh = 1 << r
                n = C - sh
                src = sT if r % 2 == 0 else sT2
                dst_ = sT2 if r % 2 == 0 else sT
                sw = [spool.tile([128, MEM], f32, tag="scanw", name=f"sw{r}_{_i}") for _i in range(2)]
                for kp in range(2):
                    nc.sync.dma_start(rr(sw[kp][:]), rr(scan_in[r, kp * 128:(kp + 1) * 128, :]))
                for b in range(NB):
                    for kt in range(2):
                        ps = pm.tile([128, C], f32, tag="pm")
                        for kp in range(2):
                            nc.tensor.matmul(
                                ps[:], sw[kp][:, kt * 128:(kt + 1) * 128].bitcast(f32r),
                                src[kp][:, b * C:b * C + C].bitcast(f32r),
                                start=(kp == 0), stop=(kp == 1))
                        nc.vector.tensor_copy(rr(dst_[kt][:, b * C:b * C + sh]),
                                              src[kt][:, b * C:b * C + sh])
                        nc.vector.tensor_add(rr(dst_[kt][:, b * C + sh:(b + 1) * C]),
                                             ps[:, 0:n], src[kt][:, b * C + sh:(b + 1) * C])

            # ---- phase 5: exclusive shift ----
            for kt in range(2 if max_phase >= 5 else 0):
                for b in range(NB):
                    nc.vector.tensor_scalar_mul(rr(sTx[kt][:, b * C:b * C + 1]),
                                                sT2[kt][:, b * C:b * C + 1], 0.0)
                    nc.vector.tensor_copy(rr(sTx[kt][:, b * C + 1:(b + 1) * C]),
                                          sT2[kt][:, b * C:(b + 1) * C - 1])

            # ---- phase 6: m = W_big^T @ [s_excl; u_c] ----
            for g in range(16 if max_phase >= 6 else 0):          # groups of 2 i-blocks (512 cols)
                wb = [wpool.tile([128, 512], f32, tag="wbig", name=f"wb{g}_{_i}") for _i in range(2)]
                for kp in range(2):
                    nc.sync.dma_start(
                        rr(wb[kp][:]), rr(wbig_in[kp * 128:(kp + 1) * 128,
                                                  g * 512:(g + 1) * 512]))
                wb2 = w2pool.tile([LT, 512], f32, tag="wbig2")
                nc.sync.dma_start(rr(wb2[:]), rr(wbig_in[MEM:MEM + LT, g * 512:(g + 1) * 512]))
                for sub in range(4):
                    i = 2 * g + sub // 2
                    kh = sub % 2
                    cs = sub * 128
                    ps = pm.tile([128, NB * C], f32, tag="pm")
                    nc.tensor.matmul(ps[:], wb[0][:, cs:cs + 128].bitcast(f32r),
                                     sTx[0][:].bitcast(f32r), start=True, stop=False)
                    nc.tensor.matmul(ps[:], wb[1][:, cs:cs + 128].bitcast(f32r),
                                     sTx[1][:].bitcast(f32r), start=False, stop=False)
                    nc.tensor.matmul(ps[:], wb2[:, cs:cs + 128].bitcast(f32r),
                                     uT[:].bitcast(f32r), start=False, stop=True)
                    if CFG["mergeevac"]:
                        dst = mTj[kh][:].rearrange(
                            "p (b c i) -> p b c i", b=NB, i=LT)[:, :, :, i]
                        nc.vector.tensor_copy(rr(dst),
                                              ps[:].rearrange("p (b c) -> p b c", b=NB))
                    else:
                        for b in range(NB):
                            dst = mT[b][kh][:].rearrange("p (c i) -> p c i", i=LT)[:, :, i]
                            nc.vector.tensor_copy(rr(dst), ps[:, b * C:(b + 1) * C])

            # ---- phase 7: h = relu([m, x] @ Wh^T + bias) ----
            for b in range(NB if max_phase >= 7 else 0):
                for t in range(TT):
                    ph = pf.tile([128, HID], f32, tag="pf")
                    ts = slice(t * 128, (t + 1) * 128)
                    if CFG["mergeevac"]:
                        bts = slice(b * S + t * 128, b * S + (t + 1) * 128)
                        lhs_aps = [mTj[0][:, bts], mTj[1][:, bts],
                                   xT[b][0][:, ts], xT[b][1][:, ts]]
                    else:
                        lhs_aps = [mT[b][0][:, ts], mT[b][1][:, ts],
                                   xT[b][0][:, ts], xT[b][1][:, ts]]
                    for k in range(4):
                        nc.tensor.matmul(ph[:], lhs_aps[k].bitcast(f32r),
                                         wh_sb[k][:].bitcast(f32r),
                                         start=(k == 0), stop=(k == 3))
                    hs = hpool.tile([128, HID], f32, tag="hout")
                    nc.vector.scalar_tensor_tensor(
                        hs[:], ph[:], 1.0, bias_sb[:],
                        op0=mybir.AluOpType.mult, op1=mybir.AluOpType.add)
                    nc.scalar.activation(hs[:], hs[:],
                                         mybir.ActivationFunctionType.Relu)
                    nc.sync.dma_start(h_out[b, ts, :], hs[:])
    nc.compile()
    return nc


def kernel(x, Wu_w, Wu_b, Wh_w, Wh_b, H, _profile=False):
    from concourse.bass_utils import run_bass_kernel_spmd

    x = np.ascontiguousarray(np.asarray(x, dtype=np.float32))
    H = np.asarray(H, dtype=np.float32)
    W_big, scanT, Hrev32 = _host_consts(H)
    whcat = np.ascontiguousarray(np.asarray(Wh_w, np.float32).T)
    biasb = np.tile(np.asarray(Wh_b, np.float32)[None, :], (128, 1))
    wu = np.ascontiguousarray(np.asarray(Wu_w, np.float32).T)
    wub = np.asarray(Wu_b, np.float32).reshape(1, 1)

    if 'nc' not in _CACHE:
        _CACHE['nc'] = _build_program()
    nc = _CACHE['nc']

    in_maps = []
    for c in range(NCORES):
        in_maps.append({
            "x_in": x[c * NB:(c + 1) * NB],
            "wbig": W_big, "scan7": scanT, "hrev": Hrev32,
            "whcat": whcat, "biasb": biasb, "wu": wu, "wub": wub,
        })
    res = run_bass_kernel_spmd(nc, in_maps, list(range(NCORES)), trace=False)
    h = np.concatenate([res.results[c]["h_out"] for c in range(NCORES)], axis=0)
    return h, h[:, -1, :]


# revision 19
# speedup vs baseline: 1.0614x; 1.0614x over previous
"""LMU-FFT cell as a Bass/Tile kernel on 8 trn2 NeuronCores.

Replaces the reference FFT convolution with an exact chunked state-space
form (validated to ~3e-7 rel err vs the FFT on CPU):
  u = relu(x @ Wu^T + b)                          [B, S]
  chunks of Lt=32: p[c] = Hrev @ u_c; boundary states via log-depth
  matrix scan s[c] = sum_d Ad^(32(c-d)) p[d]; then
  m[c] = W_big^T @ [s[c-1]; u_c]  where W_big packs Ad^(i+1) and the
  local Toeplitz H columns; finally h = relu([m, x] @ Wh^T + bias).
Data-parallel over batch: 2 batch rows per core, all params replicated.
"""
import numpy as np

B, S, I, MEM, HID = 16, 4096, 256, 256, 512
LT = 32
C = S // LT            # 128 chunks per sequence
NB = 2                 # batches per core
NCORES = 8
THETA = float(S)

_CACHE = {}
CFG = {"pt": 2, "pm": 3, "pf": 3, "wp": 8, "w2": 4, "xp": 3, "hp": 6, "sp": 4, "xbatch": True, "mergeevac": True, "actsplit": False}


def _state_space_f32():
    import jax
    import jax.numpy as jnp
    from jax.scipy.linalg import expm
    n = MEM
    Q = np.arange(n, dtype=np.float64).reshape(-1, 1)
    R = (2 * Q + 1) / THETA
    i, j = np.meshgrid(Q, Q, indexing='ij')
    A = R * np.where(i < j, -1.0, (-1.0) ** (i - j + 1))
    Bm = R * (-1.0) ** Q
    M = np.zeros((n + 1, n + 1), dtype=np.float32)
    M[:n, :n] = A.astype(np.float32)
    M[:n, n:] = Bm.astype(np.float32)
    with jax.default_device(jax.devices('cpu')[0]):
        Md = np.array(expm(jnp.asarray(M)))
    return Md[:n, :n]


def _host_consts(H):
    Ad64 = _state_space_f32().astype(np.float64)
    G = np.empty((LT, MEM, MEM), dtype=np.float64)
    P = Ad64.copy()
    for i in range(LT):
        if i > 0:
            P = P @ Ad64
        G[i] = P
    scanT = np.empty((7, MEM, MEM), dtype=np.float32)
    Q = np.linalg.matrix_power(Ad64, LT)
    for r in range(7):
        scanT[r] = Q.T.astype(np.float32)
        Q = Q @ Q
    W_big = np.zeros((MEM + LT, LT * MEM), dtype=np.float32)
    for i in range(LT):
        W_big[:MEM, i * MEM:(i + 1) * MEM] = G[i].T.astype(np.float32)
        for jj in range(i + 1):
            W_big[MEM + jj, i * MEM:(i + 1) * MEM] = H[:, i - jj]
    Hrev32 = np.ascontiguousarray(H[:, :LT][:, ::-1].T)   # [LT, MEM]
    return W_big, scanT, Hrev32


def _build_program(max_phase=99):
    import concourse.bass as bass
    import concourse.mybir as mybir
    import concourse.tile as tile
    from concourse import bacc
    from concourse.masks import make_identity

    f32 = mybir.dt.float32
    f32r = mybir.dt.float32r
    USE_F32R = True

    def rr(ap):
        return ap.bitcast(f32r) if USE_F32R else ap
    nc = bacc.Bacc("TRN2", target_bir_lowering=False, debug=False)

    x_in = nc.declare_dram_parameter("x_in", [NB, S, I], f32, isOutput=False)
    wbig_in = nc.declare_dram_parameter("wbig", [MEM + LT, LT * MEM], f32, isOutput=False)
    scan_in = nc.declare_dram_parameter("scan7", [7, MEM, MEM], f32, isOutput=False)
    hrev_in = nc.declare_dram_parameter("hrev", [LT, MEM], f32, isOutput=False)
    whcat_in = nc.declare_dram_parameter("whcat", [HID, HID], f32, isOutput=False)
    bias_in = nc.declare_dram_parameter("biasb", [128, HID], f32, isOutput=False)
    wu_in = nc.declare_dram_parameter("wu", [I, 1], f32, isOutput=False)
    wub_in = nc.declare_dram_parameter("wub", [1, 1], f32, isOutput=False)
    h_out = nc.declare_dram_parameter("h_out", [NB, S, HID], f32, isOutput=True)
    u_dram = nc.dram_tensor("u_scratch", [NB, S], f32)

    TT = S // 128        # 32 time-tiles per batch

    with tile.TileContext(nc) as tc:
        with (
            tc.tile_pool(name="const", bufs=1) as cpool,
            tc.tile_pool(name="big", bufs=1) as bigpool,
            tc.tile_pool(name="xin", bufs=CFG["xp"]) as xpool,
            tc.tile_pool(name="wbig", bufs=CFG["wp"]) as wpool,
            tc.tile_pool(name="wbig2", bufs=CFG["w2"]) as w2pool,
            tc.tile_pool(name="scanw", bufs=CFG["sp"]) as spool,
            tc.tile_pool(name="hout", bufs=CFG["hp"]) as hpool,
            tc.tile_pool(name="utmp", bufs=4) as upool,
            tc.tile_pool(name="pt", bufs=CFG["pt"], space="PSUM") as pt,
            tc.tile_pool(name="pm", bufs=CFG["pm"], space="PSUM") as pm,
            tc.tile_pool(name="pf", bufs=CFG["pf"], space="PSUM") as pf,
        ):
            # ---- constants in SBUF ----
            ident = cpool.tile([128, 128], f32, tag="ident")
            make_identity(nc, ident[:])
            hrev_sb = cpool.tile([LT, MEM], f32, tag="hrev")
            nc.sync.dma_start(rr(hrev_sb[:]), rr(hrev_in[:]))
            wh_sb = [cpool.tile([128, HID], f32, tag=f"wh{k}", name=f"wh{k}") for k in range(4)]
            for k in range(4):
                nc.sync.dma_start(rr(wh_sb[k][:]), rr(whcat_in[k * 128:(k + 1) * 128, :]))
            bias_sb = cpool.tile([128, HID], f32, tag="bias")
            nc.sync.dma_start(bias_sb[:], bias_in[:])
            wu_sb = [cpool.tile([128, 1], f32, tag=f"wu{k}", name=f"wu{k}") for k in range(2)]
            for k in range(2):
                nc.sync.dma_start(rr(wu_sb[k][:]), rr(wu_in[k * 128:(k + 1) * 128, :]))
            wub_sb = cpool.tile([1, 1], f32, tag="wub")
            nc.sync.dma_start(wub_sb[:], wub_in[:])

            # persistent big buffers
            xT = [[bigpool.tile([128, S], f32, tag=f"xT{b}{fh}", name=f"xT{b}{fh}") for fh in range(2)]
                  for b in range(NB)]
            mT = [[bigpool.tile([128, S], f32, tag=f"mT{b}{kh}", name=f"mT{b}{kh}") for kh in range(2)]
                  for b in range(NB)] if not CFG["mergeevac"] else None
            mTj = [bigpool.tile([128, NB * S], f32, tag=f"mTj{kh}", name=f"mTj{kh}") for kh in range(2)] if CFG["mergeevac"] else None
            sT = [bigpool.tile([128, NB * C], f32, tag=f"sT{k}", name=f"sT{k}") for k in range(2)]
            sT2 = [bigpool.tile([128, NB * C], f32, tag=f"sT2{k}", name=f"sT2{k}") for k in range(2)]
            sTx = [bigpool.tile([128, NB * C], f32, tag=f"sTx{k}", name=f"sTx{k}") for k in range(2)]
            uT = bigpool.tile([LT, NB * C], f32, tag="uT")
            ucj = [bigpool.tile([128, LT], f32, tag=f"ucj{b}", name=f"ucj{b}") for b in range(NB)]

            # ---- phase 1: load x, build xT via PE transpose ----
            XB = 4 if CFG["xbatch"] else 1     # t-tiles per x DMA
            for b in range(NB):
                for tg in range(TT // XB):
                    xt = xpool.tile([128, XB * I], f32, tag="xin")
                    nc.sync.dma_start(
                        xt[:].rearrange("p (tt f) -> p tt f", tt=XB),
                        x_in[b, tg * XB * 128:(tg + 1) * XB * 128, :]
                        .rearrange("(tt p) f -> p tt f", p=128))
                    for tt in range(XB):
                        t = tg * XB + tt
                        for fh in range(2):
                            ps = pt.tile([128, 128], f32, tag="pt")
                            nc.tensor.transpose(
                                ps[:], xt[:, tt * I + fh * 128:tt * I + (fh + 1) * 128],
                                ident[:])
                            if CFG["actsplit"] and fh == 1:
                                nc.scalar.copy(rr(xT[b][fh][:, t * 128:(t + 1) * 128]), ps[:])
                            else:
                                nc.vector.tensor_copy(rr(xT[b][fh][:, t * 128:(t + 1) * 128]), ps[:])

            # ---- phase 2: u = relu(x @ Wu + b), reshape to uT [32, 256] ----
            for b in range(NB if max_phase >= 2 else 0):
                for q in range(S // 512):
                    pu = pf.tile([1, 512], f32, tag="pf")
                    for fh in range(2):
                        nc.tensor.matmul(
                            pu[:], wu_sb[fh][:].bitcast(f32r),
                            xT[b][fh][:, q * 512:(q + 1) * 512].bitcast(f32r),
                            start=(fh == 0), stop=(fh == 1))
                    ut = upool.tile([1, 512], f32, tag="utmp")
                    nc.scalar.activation(ut[:], pu[:],
                                         mybir.ActivationFunctionType.Relu,
                                         bias=wub_sb[0:1, 0:1])
                    nc.sync.dma_start(u_dram[b, q * 512:(q + 1) * 512].rearrange("(a w) -> a w", a=1), ut[0:1, :])
                for _ in range(1):
                    nc.sync.dma_start(
                        ucj[b][:], u_dram[b, :].rearrange("(c j) -> c j", j=LT))
                ps = pt.tile([128, 128], f32, tag="pt")
                nc.tensor.transpose(ps[0:LT, 0:128], ucj[b][:], ident[:])
                nc.vector.tensor_copy(rr(uT[:, b * C:(b + 1) * C]), ps[0:LT, 0:128])

            # ---- phase 3: p init: sT[kh] = Hrev^T @ uT ----
            for kh in range(2 if max_phase >= 3 else 0):
                ps = pm.tile([128, NB * C], f32, tag="pm")
                nc.tensor.matmul(ps[:], hrev_sb[:, kh * 128:(kh + 1) * 128].bitcast(f32r),
                                 uT[:].bitcast(f32r), start=True, stop=True)
                nc.vector.tensor_copy(rr(sT[kh][:]), ps[:])

            # ---- phase 4: Hillis-Steele scan over chunks (7 rounds) ----
            for r in range(7 if max_phase >= 4 else 0):
                sh = 1 << r
                n = C - sh
                src = sT if r % 2 == 0 else sT2
                dst_ = sT2 if r % 2 == 0 else sT
                sw = [spool.tile([128, MEM], f32, tag="scanw", name=f"sw{r}_{_i}") for _i in range(2)]
                for kp in range(2):
                    nc.sync.dma_start(rr(sw[kp][:]), rr(scan_in[r, kp * 128:(kp + 1) * 128, :]))
                for b in range(NB):
                    for kt in range(2):
                        ps = pm.tile([128, C], f32, tag="pm")
                        for kp in range(2):
                            nc.tensor.matmul(
                                ps[:], sw[kp][:, kt * 128:(kt + 1) * 128].bitcast(f32r),
                                src[kp][:, b * C:b * C + C].bitcast(f32r),
                                start=(kp == 0), stop=(kp == 1))
                        nc.vector.tensor_copy(rr(dst_[kt][:, b * C:b * C + sh]),
                                              src[kt][:, b * C:b * C + sh])
                        nc.vector.tensor_add(rr(dst_[kt][:, b * C + sh:(b + 1) * C]),
                                             ps[:, 0:n], src[kt][:, b * C + sh:(b + 1) * C])

            # ---- phase 5: exclusive shift ----
            for kt in range(2 if max_phase >= 5 else 0):
                for b in range(NB):
                    nc.vector.tensor_scalar_mul(rr(sTx[kt][:, b * C:b * C + 1]),
                                                sT2[kt][:, b * C:b * C + 1], 0.0)
                    nc.vector.tensor_copy(rr(sTx[kt][:, b * C + 1:(b + 1) * C]),
                                          sT2[kt][:, b * C:(b + 1) * C - 1])

            # ---- phase 6: m = W_big^T @ [s_excl; u_c] ----
            for g in range(16 if max_phase >= 6 else 0):          # groups of 2 i-blocks (512 cols)
                wb = [wpool.tile([128, 512], f32, tag="wbig", name=f"wb{g}_{_i}") for _i in range(2)]
                for kp in range(2):
                    nc.sync.dma_start(
                        rr(wb[kp][:]), rr(wbig_in[kp * 128:(kp + 1) * 128,
                                                  g * 512:(g + 1) * 512]))
                wb2 = w2pool.tile([LT, 512], f32, tag="wbig2")
                nc.sync.dma_start(rr(wb2[:]), rr(wbig_in[MEM:MEM + LT, g * 512:(g + 1) * 512]))
                for sub in range(4):
                    i = 2 * g + sub // 2
                    kh = sub % 2
                    cs = sub * 128
                    ps = pm.tile([128, NB * C], f32, tag="pm")
                    nc.tensor.matmul(ps[:], wb[0][:, cs:cs + 128].bitcast(f32r),
                                     sTx[0][:].bitcast(f32r), start=True, stop=False)
                    nc.tensor.matmul(ps[:], wb[1][:, cs:cs + 128].bitcast(f32r),
                                     sTx[1][:].bitcast(f32r), start=False, stop=False)
                    nc.tensor.matmul(ps[:], wb2[:, cs:cs + 128].bitcast(f32r),
                                     uT[:].bitcast(f32r), start=False, stop=True)
                    if CFG["mergeevac"]:
                        dst = mTj[kh][:].rearrange(
                            "p (b c i) -> p b c i", b=NB, i=LT)[:, :, :, i]
                        nc.vector.tensor_copy(rr(dst),
                                              ps[:].rearrange("p (b c) -> p b c", b=NB))
                    else:
                        for b in range(NB):
                            dst = mT[b][kh][:].rearrange("p (c i) -> p c i", i=LT)[:, :, i]
                            nc.vector.tensor_copy(rr(dst), ps[:, b * C:(b + 1) * C])

            # ---- phase 7: h = relu([m, x] @ Wh^T + bias) ----
            for b in range(NB if max_phase >= 7 else 0):
                for t in range(TT):
                    ph = pf.tile([128, HID], f32, tag="pf")
                    ts = slice(t * 128, (t + 1) * 128)
                    if CFG["mergeevac"]:
                        bts = slice(b * S + t * 128, b * S + (t + 1) * 128)
                        lhs_aps = [mTj[0][:, bts], mTj[1][:, bts],
                                   xT[b][0][:, ts], xT[b][1][:, ts]]
                    else:
                        lhs_aps = [mT[b][0][:, ts], mT[b][1][:, ts],
                                   xT[b][0][:, ts], xT[b][1][:, ts]]
                    for k in range(4):
                        nc.tensor.matmul(ph[:], lhs_aps[k].bitcast(f32r),
                                         wh_sb[k][:].bitcast(f32r),
                                         start=(k == 0), stop=(k == 3))
                    hs = hpool.tile([128, HID], f32, tag="hout")
                    nc.vector.scalar_tensor_tensor(
                        hs[:], ph[:], 1.0, bias_sb[:],
                        op0=mybir.AluOpType.mult, op1=mybir.AluOpType.add)
                    nc.scalar.activation(hs[:], hs[:],
                                         mybir.ActivationFunctionType.Relu)
                    nc.sync.dma_start(h_out[b, ts, :], hs[:])
    nc.compile()
    return nc


def kernel(x, Wu_w, Wu_b, Wh_w, Wh_b, H, _profile=False):
    from concourse.bass_utils import run_bass_kernel_spmd

    x = np.ascontiguousarray(np.asarray(x, dtype=np.float32))
    H = np.asarray(H, dtype=np.float32)
    W_big, scanT, Hrev32 = _host_consts(H)
    whcat = np.ascontiguousarray(np.asarray(Wh_w, np.float32).T)
    biasb = np.tile(np.asarray(Wh_b, np.float32)[None, :], (128, 1))
    wu = np.ascontiguousarray(np.asarray(Wu_w, np.float32).T)
    wub = np.asarray(Wu_b, np.float32).reshape(1, 1)

    if 'nc' not in _CACHE:
        _CACHE['nc'] = _build_program()
    nc = _CACHE['nc']

    in_maps = []
    for c in range(NCORES):
        in_maps.append({
            "x_in": x[c * NB:(c + 1) * NB],
            "wbig": W_big, "scan7": scanT, "hrev": Hrev32,
            "whcat": whcat, "biasb": biasb, "wu": wu, "wub": wub,
        })
    res = run_bass_kernel_spmd(nc, in_maps, list(range(NCORES)), trace=False)
    h = np.concatenate([res.results[c]["h_out"] for c in range(NCORES)], axis=0)
    return h, h[:, -1, :]
